# revision 22
# baseline (speedup 1.0000x reference)
"""Trainium2 Bass kernel for nn_ClearMeshLoss (8-core SPMD).

Strategy:
  - chamfer + normal-consistency: ONE c' = -d matrix per core (pred rows x
    all gt cols), via a K=5 augmented matmul that folds both -|g|^2 and
    -|p|^2: c'_ij = 2 p.g - |g|^2 - |p|^2 = -d_ij.  Then
      A-side min_j d_ij = -rowmax(c')      (DVE fold-tree on f16 copy)
      B-side min_i d_ij = -colmax(c')      (Pool running elementwise max,
                                            final partition reduce via PE
                                            transposes + DVE reduces)
      argmin_j for normal consistency      (DVE is_ge x iota accum scan)
    This halves the fundamental work vs computing both pred x gt and
    gt x pred matrices.  Matmuls run in float32r (1 cyc/row vs 4 for fp32).
  - sdf/eikonal: data-parallel over the flattened 200000 elements.
  - edge loss: host does integer edge pairing; device does the float work.
  - host combines tiny per-core partials into the final scalar.
"""
import numpy as np

# ---------------------------------------------------------------- constants
SDF_W, EIK_W, CH_W, NORM_W, EDGE_W, WT_W = 1.0, 0.1, 1.0, 0.5, 0.3, 0.2
TRUNC, SURF_W, DIH_THR = 0.1, 5.0, 0.5
SIGMA = TRUNC / 3.0

N_CORES = 8
FAR = 100.0          # pad-point coordinate; keeps |c'| within f16 range

# full-size problem config (hardcoded from the problem spec)
FULL_CFG = dict(
    npts=10000,          # points per cloud
    rows_pad=1280,       # per-core padded row count (10 strips of 128)
    cols_pad=10240,      # padded column count (streamed side)
    super_w=2048,        # PSUM supertile width (4 banks)
    tile_w=512,          # matmul free dim
    mm_dtype="float32r", # matmul input dtype (float32r = 4x faster than fp32)
    sdf_n=200000,        # total sdf elements (B*N)
    sdf_shard=25000,     # per-core sdf elements
    sdf_f=196,           # sdf tile free dim ([128,196] = 25088 >= 25000)
    eik_f=196,           # eikonal diffs per partition row
    pair_cap=122880,     # total edge-pair capacity (8*128*120)
    pair_f=120,          # per-core edge pair tile free dim
)

_PROG_CACHE = {}


def build_program(cfg, phases=("cham", "sdf", "eik", "edge")):
    """Build the (single-core SPMD) Bass/Tile program for one config."""
    from contextlib import ExitStack
    import concourse.bacc as bacc
    import concourse.bass as bass
    import concourse.tile as tile
    from concourse import mybir

    f32 = mybir.dt.float32
    f16 = mybir.dt.float16
    fmm = getattr(mybir.dt, cfg["mm_dtype"])
    AX = mybir.AxisListType
    OP = mybir.AluOpType
    AF = mybir.ActivationFunctionType

    rows_pad = cfg["rows_pad"]
    cols_pad = cfg["cols_pad"]
    super_w = cfg["super_w"]
    tile_w = cfg["tile_w"]
    sdf_f = cfg["sdf_f"]
    eik_f = cfg["eik_f"]
    pair_f = cfg["pair_f"]

    n_strips = rows_pad // 128
    n_super = cols_pad // super_w
    mm_per_super = super_w // tile_w

    nc = bacc.Bacc("TRN2", target_bir_lowering=False)

    # ---- inputs (per-core values supplied by host) ----
    d_a_pred = nc.dram_tensor("a_pred", [5, rows_pad], fmm, kind="ExternalInput")
    d_b_gt = nc.dram_tensor("b_gt", [5, cols_pad], fmm, kind="ExternalInput")
    d_sdf_pred = nc.dram_tensor("sdf_pred", [128, sdf_f], f32, kind="ExternalInput")
    d_sdf_gt = nc.dram_tensor("sdf_gt", [128, sdf_f], f32, kind="ExternalInput")
    d_eik_pred = nc.dram_tensor("eik_pred", [128 * eik_f + 1], f32, kind="ExternalInput")
    d_eik_gt = nc.dram_tensor("eik_gt", [128, eik_f], f32, kind="ExternalInput")
    d_edge = nc.dram_tensor("edge_in", [18, 128, pair_f], f32, kind="ExternalInput")

    # ---- outputs ----
    # cham_out columns: [0..ns) A rowmax(c'), [ns..2ns) A argmax idx
    d_cham = nc.dram_tensor("cham_out", [128, 2 * n_strips], f32, kind="ExternalOutput")
    # colmax partials, laid out [col_within_chunk(128), chunk(cols_pad/128)]
    d_colmax = nc.dram_tensor("colmax_out", [128, cols_pad // 128], f32,
                              kind="ExternalOutput")
    # part_out cols: 0 sdf_absdiff, 1 sdf_4e_absdiff, 2 eik_num, 3 eik_cnt, 4 edge_relu
    d_part = nc.dram_tensor("part_out", [128, 8], f32, kind="ExternalOutput")

    with tile.TileContext(nc) as tc, ExitStack() as octx:
        singles = octx.enter_context(tc.tile_pool(name="singles", bufs=1))
        cham_o = singles.tile([128, 2 * n_strips], f32)
        colmax_o = singles.tile([128, cols_pad // 128], f32)
        part_o = singles.tile([128, 8], f32)
        nc.vector.memset(part_o, 0.0)

        # ======== sdf + eikonal + edge first: their DVE work overlaps the
        # ======== chamfer pipeline fill (PE matmuls + first Act copies)
        with ExitStack() as ctx:
            spool = ctx.enter_context(tc.tile_pool(name="spool", bufs=1))
            if "sdf" not in phases:
                nc.vector.memset(part_o[:, 0:2], 0.0)
            if "eik" not in phases:
                nc.vector.memset(part_o[:, 2:3], 0.0)
                nc.vector.memset(part_o[:, 3:4], 1.0)
            if "sdf" in phases:
                _emit_sdf(nc, spool, part_o, d_sdf_pred, d_sdf_gt, sdf_f, f32, AX, OP, AF)
            if "eik" in phases:
                _emit_eik(nc, bass, spool, part_o, d_eik_pred, d_eik_gt, eik_f, f32, AX, OP, AF)
        if "edge" not in phases:
            nc.vector.memset(part_o[:, 4:5], 0.0)
        if "edge" in phases:
          with ExitStack() as ctx:
            epool = ctx.enter_context(tc.tile_pool(name="epool", bufs=1))
            _emit_edge(nc, bass, epool, part_o, d_edge, pair_f, f32, AX, OP, AF)

        # ================= chamfer / normal consistency =================
        if "cham" not in phases:
            nc.vector.memset(cham_o, 0.0)
            nc.vector.memset(colmax_o, 0.0)
        if "cham" in phases:
          with ExitStack() as ctx:
            cpool = ctx.enter_context(tc.tile_pool(name="cpool", bufs=1))
            strips = ctx.enter_context(tc.tile_pool(name="strips", bufs=2))
            lhsp = ctx.enter_context(tc.tile_pool(name="lhsp", bufs=2))
            foldp = ctx.enter_context(tc.tile_pool(name="foldp", bufs=1))

            b_gt_t = cpool.tile([5, cols_pad], fmm)
            nc.sync.dma_start(out=b_gt_t, in_=d_b_gt[:, :])

            iota_t = cpool.tile([128, cols_pad], f32)
            nc.gpsimd.iota(out=iota_t[:, :], pattern=[[1, cols_pad]], base=0,
                           channel_multiplier=0,
                           allow_small_or_imprecise_dtypes=True)

            run_t = cpool.tile([128, cols_pad], f16)
            nc.vector.memset(run_t, -60000.0)

            # identity matrix for PE transposes: id[p, j] = (j == p)
            id_t = cpool.tile([128, 128], f16)
            iota_p = cpool.tile([128, 1], f32)
            nc.gpsimd.iota(out=iota_p[:, :], pattern=[[1, 1]], base=0,
                           channel_multiplier=1,
                           allow_small_or_imprecise_dtypes=True)
            nc.vector.tensor_scalar(out=id_t, in0=iota_t[:, 0:128],
                                    scalar1=iota_p[:, 0:1], scalar2=None,
                                    op0=OP.is_equal)

            scratch_t = cpool.tile([128, cols_pad], f32)
            scratch = scratch_t[:, :]

            mm_ctx = ExitStack()
            psum = mm_ctx.enter_context(tc.tile_pool(name="psum", bufs=2, space="PSUM"))
            for s in range(n_strips):
                lhsA = lhsp.tile([5, 128], fmm, tag="lhs")
                nc.sync.dma_start(out=lhsA, in_=d_a_pred[:, s * 128:(s + 1) * 128])
                stripA = strips.tile([128, cols_pad], f16, tag="strip")
                for c in range(n_super):
                    ps = psum.tile([128, super_w], f32, tag="ps")
                    for m in range(mm_per_super):
                        lo = c * super_w + m * tile_w
                        nc.tensor.matmul(ps[:, m * tile_w:(m + 1) * tile_w],
                                         lhsA[:, :], b_gt_t[:, lo:lo + tile_w],
                                         start=True, stop=True)
                    # evacuate PSUM to f16 strip (Act engine)
                    nc.scalar.activation(out=stripA[:, c * super_w:(c + 1) * super_w],
                                         in_=ps[:, :], func=AF.Copy)

                # ---- rowmax via fold tree on f16 (DVE, 2x mode) ----
                fa = foldp.tile([128, cols_pad // 2], f16, tag="fa")
                fb = foldp.tile([128, cols_pad // 4], f16, tag="fb")
                w = cols_pad // 2
                nc.vector.tensor_tensor(out=fa[:, :w], in0=stripA[:, :w],
                                        in1=stripA[:, w:], op=OP.max)
                cur, nxt = fa, fb
                while w > 256:
                    h = w // 2
                    nc.vector.tensor_tensor(out=nxt[:, :h], in0=cur[:, :h],
                                            in1=cur[:, h:w], op=OP.max)
                    cur, nxt = nxt, cur
                    w = h
                rmaxA = cham_o[:, s:s + 1]
                nc.vector.tensor_reduce(out=rmaxA, in_=cur[:, :w], axis=AX.X,
                                        op=OP.max)

                # ---- argmax scan (DVE stt: is_ge(rowmax) * iota, sum) ----
                nc.vector.scalar_tensor_tensor(
                    out=scratch, in0=stripA[:, :], scalar=rmaxA,
                    in1=iota_t[:, :], op0=OP.is_ge, op1=OP.mult,
                    accum_out=cham_o[:, n_strips + s:n_strips + s + 1])

                # ---- running colmax (DVE, f16 2x mode) ----
                nc.vector.tensor_tensor(out=run_t, in0=stripA, in1=run_t,
                                        op=OP.max)

            mm_ctx.close()

            # ---- final colmax partition-reduce: PE transpose + DVE reduce ----
            n_chunk = cols_pad // 128
            with ExitStack() as ctx2:
                tpsum = ctx2.enter_context(tc.tile_pool(name="tpsum", bufs=2,
                                                        space="PSUM"))
                for g in range(0, n_chunk, 8):
                    tp = tpsum.tile([128, 8 * 128], f16, tag="tp")
                    for k in range(8):
                        nc.tensor.transpose(
                            tp[:, k * 128:(k + 1) * 128],
                            run_t[:, (g + k) * 128:(g + k + 1) * 128],
                            id_t[:, :])
                    src = bass.AP(tensor=tp.tensor, offset=tp[:, :].offset,
                                  ap=[[tp[:, :].ap[0][0], 128], [128, 8], [1, 128]])
                    nc.vector.tensor_reduce(out=colmax_o[:, g:g + 8], in_=src,
                                            axis=AX.X, op=OP.max)

        nc.sync.dma_start(out=d_cham[:, :], in_=cham_o[:, :])
        nc.sync.dma_start(out=d_colmax[:, :], in_=colmax_o[:, :])
        nc.sync.dma_start(out=d_part[:, :], in_=part_o[:, :])

    nc.compile()
    return nc


def _emit_sdf(nc, spool, part_o, d_sdf_pred, d_sdf_gt, sdf_f, f32, AX, OP, AF):
        if True:
            pr = spool.tile([128, sdf_f], f32)
            g = spool.tile([128, sdf_f], f32)
            nc.sync.dma_start(out=pr, in_=d_sdf_pred[:, :])
            nc.sync.dma_start(out=g, in_=d_sdf_gt[:, :])

            prc = spool.tile([128, sdf_f], f32)
            gc = spool.tile([128, sdf_f], f32)
            nc.vector.tensor_scalar(out=prc, in0=pr, scalar1=TRUNC, scalar2=-TRUNC,
                                    op0=OP.min, op1=OP.max)
            nc.vector.tensor_scalar(out=gc, in0=g, scalar1=TRUNC, scalar2=-TRUNC,
                                    op0=OP.min, op1=OP.max)
            diff = spool.tile([128, sdf_f], f32)
            nc.vector.tensor_tensor(out=diff, in0=prc, in1=gc, op=OP.subtract)
            absdiff = spool.tile([128, sdf_f], f32)
            nc.scalar.activation(out=absdiff, in_=diff, func=AF.Abs,
                                 accum_out=part_o[:, 0:1])
            absg = spool.tile([128, sdf_f], f32)
            nc.scalar.activation(out=absg, in_=gc, func=AF.Abs)
            e = spool.tile([128, sdf_f], f32)
            nc.scalar.activation(out=e, in_=absg, func=AF.Exp, scale=-1.0 / SIGMA)
            dead = spool.tile([128, sdf_f], f32)
            nc.vector.scalar_tensor_tensor(out=dead, in0=e, scalar=SURF_W - 1.0,
                                           in1=absdiff, op0=OP.mult, op1=OP.mult,
                                           accum_out=part_o[:, 1:2])


def _emit_eik(nc, bass, spool, part_o, d_eik_pred, d_eik_gt, eik_f, f32, AX, OP, AF):
        if True:
            # eikonal: two shifted (non-overlapping within themselves) loads
            ep0 = spool.tile([128, eik_f], f32)
            ep1 = spool.tile([128, eik_f], f32)
            base = d_eik_pred[:]
            src0 = bass.AP(tensor=base.tensor, offset=0,
                           ap=[[eik_f, 128], [1, eik_f]])
            src1 = bass.AP(tensor=base.tensor, offset=1,
                           ap=[[eik_f, 128], [1, eik_f]])
            nc.sync.dma_start(out=ep0[:, :], in_=src0)
            nc.sync.dma_start(out=ep1[:, :], in_=src1)
            eg = spool.tile([128, eik_f], f32)
            nc.sync.dma_start(out=eg, in_=d_eik_gt[:, :])

            dx = spool.tile([128, eik_f], f32)
            nc.vector.tensor_tensor(out=dx, in0=ep1[:, :],
                                    in1=ep0[:, :], op=OP.subtract)
            absdx = spool.tile([128, eik_f], f32)
            nc.scalar.activation(out=absdx, in_=dx, func=AF.Abs)
            t = spool.tile([128, eik_f], f32)
            nc.vector.tensor_scalar(out=t, in0=absdx, scalar1=-1.0, scalar2=None,
                                    op0=OP.add)
            t2 = spool.tile([128, eik_f], f32)
            nc.vector.tensor_tensor(out=t2, in0=t, in1=t, op=OP.mult)
            abseg = spool.tile([128, eik_f], f32)
            nc.scalar.activation(out=abseg, in_=eg, func=AF.Abs)
            mask = spool.tile([128, eik_f], f32)
            nc.vector.tensor_scalar(out=mask, in0=abseg, scalar1=TRUNC, scalar2=None,
                                    op0=OP.is_lt)
            mt2 = spool.tile([128, eik_f], f32)
            nc.vector.tensor_tensor(out=mt2, in0=t2, in1=mask, op=OP.mult)
            nc.vector.tensor_reduce(out=part_o[:, 2:3], in_=mt2, axis=AX.X,
                                    op=OP.add)
            nc.vector.tensor_reduce(out=part_o[:, 3:4], in_=mask, axis=AX.X,
                                    op=OP.add)


def _emit_edge(nc, bass, epool, part_o, d_edge, pair_f, f32, AX, OP, AF):
        if True:
            ev = epool.tile([128, 18, pair_f], f32)
            base = d_edge[0, :, :]
            for p in range(0, 18, 3):
                src3 = bass.AP(tensor=base.tensor, offset=p * 128 * pair_f,
                               ap=[[pair_f, 128], [128 * pair_f, 3], [1, pair_f]])
                nc.scalar.dma_start(out=ev[:, p:p + 3, :], in_=src3)

            # paired view of planes (p, p+9): face-A lane 0, face-B lane 1
            def pv(p):
                base = ev[:, p, :]
                return bass.AP(tensor=base.tensor, offset=base.offset,
                               ap=[[base.ap[0][0], 128], [9 * pair_f, 2],
                                   [1, pair_f]])

            def ptile():
                o = epool.tile([128, 2, pair_f], f32, name=f"ptmp_{ptile.n}")
                ptile.n += 1
                return o
            ptile.n = 0

            def tt2(op, a, b):    # paired [128,2,pair_f] op
                o = ptile()
                nc.vector.tensor_tensor(out=o[:, :, :], in0=a, in1=b, op=op)
                return o[:, :, :]

            def tt(op, a, b):     # single-lane [128,pair_f] op
                o = epool.tile([128, pair_f], f32, name=f"tmp_{tt.n}")
                tt.n += 1
                nc.vector.tensor_tensor(out=o, in0=a, in1=b, op=op)
                return o
            tt.n = 0

            # both faces' edge vectors at once
            e1 = [tt2(OP.subtract, pv(3 + i), pv(0 + i)) for i in range(3)]
            e2 = [tt2(OP.subtract, pv(6 + i), pv(0 + i)) for i in range(3)]
            # paired cross product: n = e1 x e2 for A and B lanes together
            n = [tt2(OP.subtract, tt2(OP.mult, e1[1], e2[2]),
                     tt2(OP.mult, e1[2], e2[1])),
                 tt2(OP.subtract, tt2(OP.mult, e1[2], e2[0]),
                     tt2(OP.mult, e1[0], e2[2])),
                 tt2(OP.subtract, tt2(OP.mult, e1[0], e2[1]),
                     tt2(OP.mult, e1[1], e2[0]))]
            na = [n[i][:, 0, :] for i in range(3)]
            nb = [n[i][:, 1, :] for i in range(3)]

            def dot3(a, b):
                s = tt(OP.mult, a[0], b[0])
                s = tt(OP.add, s, tt(OP.mult, a[1], b[1]))
                s = tt(OP.add, s, tt(OP.mult, a[2], b[2]))
                return s

            dot = dot3(na, nb)
            # |na|^2 and |nb|^2 in one paired chain
            nn2 = tt2(OP.mult, n[0][:, :, :], n[0][:, :, :])
            nn2 = tt2(OP.add, nn2, tt2(OP.mult, n[1][:, :, :], n[1][:, :, :]))
            nn2 = tt2(OP.add, nn2, tt2(OP.mult, n[2][:, :, :], n[2][:, :, :]))
            prod2 = tt(OP.mult, nn2[:, 0, :], nn2[:, 1, :])   # (|na| |nb|)^2
            sa = epool.tile([128, pair_f], f32)
            nc.scalar.activation(out=sa, in_=prod2, func=AF.Sqrt)
            sac = epool.tile([128, pair_f], f32)
            nc.vector.tensor_scalar(out=sac, in0=sa, scalar1=1e-24, scalar2=None,
                                    op0=OP.max)
            rs = epool.tile([128, pair_f], f32)
            nc.vector.reciprocal(out=rs, in_=sac)
            cos = tt(OP.mult, dot, rs)
            relu = epool.tile([128, pair_f], f32)
            nbias = epool.tile([128, 1], f32)
            nc.vector.memset(nbias, -DIH_THR)
            nc.scalar.activation(out=relu, in_=cos, func=AF.Relu, bias=nbias[:, 0:1],
                                 accum_out=part_o[:, 4:5])


def get_program(cfg_key="full"):
    if cfg_key not in _PROG_CACHE:
        _PROG_CACHE[cfg_key] = build_program(FULL_CFG)
    return _PROG_CACHE[cfg_key]


# ================================================================== host side
def _host_prep(inputs, cfg):
    """Build the 8 per-core input maps. Only int indexing / packing here."""
    np_f32 = np.float32
    pred_pts = np.ascontiguousarray(inputs["pred_points"][0], dtype=np_f32)  # [N,3]
    gt_pts = np.ascontiguousarray(inputs["gt_points"][0], dtype=np_f32)
    npts = cfg["npts"]
    rows_pad, cols_pad = cfg["rows_pad"], cfg["cols_pad"]
    shard = npts // N_CORES

    def a_aug(p):  # [5, n]: [p, 1, |p|^2]
        return np.concatenate([p.T, np.ones((1, p.shape[0]), np_f32),
                               (p * p).sum(-1)[None, :]], 0)

    def b_aug(p):  # [5, n]: [2g, -|g|^2, -1]
        return np.concatenate([2.0 * p.T, -(p * p).sum(-1)[None, :],
                               -np.ones((1, p.shape[0]), np_f32)], 0)

    def pad_pts(p, n):
        out = np.full((n, 3), FAR, np_f32)
        out[:p.shape[0]] = p
        return out

    b_gt_full = np.ascontiguousarray(b_aug(pad_pts(gt_pts, cols_pad)))

    # --- sdf / eikonal shards ---
    pred_sdf = inputs["pred_sdf"].reshape(-1).astype(np_f32)   # [200000]
    gt_sdf = inputs["gt_sdf"].reshape(-1).astype(np_f32)
    n_tot = pred_sdf.shape[0]
    sdf_shard, sdf_f, eik_f = cfg["sdf_shard"], cfg["sdf_f"], cfg["eik_f"]
    n_batch = inputs["pred_sdf"].shape[1]  # 100000 (seam stride)

    # --- edge pairing on host (int32 faces only) ---
    verts = np.asarray(inputs["extracted_vertices"], dtype=np_f32)
    faces = np.asarray(inputs["extracted_faces"], dtype=np.int64)
    V = verts.shape[0]
    Fn = faces.shape[0]
    a = faces
    b = np.roll(faces, -1, axis=1)
    lo = np.minimum(a, b)
    hi = np.maximum(a, b)
    key = (lo * V + hi).reshape(-1)
    fid = np.repeat(np.arange(Fn, dtype=np.int64), 3)
    order = np.argsort(key, kind="stable")
    k = key[order]
    f = fid[order]
    same_next = k[:-1] == k[1:]
    prev = np.concatenate([[False], same_next[:-1]])
    nxt = np.concatenate([same_next[1:], [False]])
    is_pair = same_next & ~prev & ~nxt
    pos = np.nonzero(is_pair)[0]
    fa = f[pos]
    fb = f[pos + 1]
    npairs = int(pos.shape[0])
    # watertight (int only)
    is_start = np.concatenate([[True], k[1:] != k[:-1]])
    starts = np.nonzero(is_start)[0]
    run_len = np.diff(np.concatenate([starts, [k.shape[0]]]))
    total_unique = int(starts.shape[0])
    bad = int((run_len != 2).sum())
    wt = (bad / total_unique) if total_unique > 0 else 0.0

    # pack pair vertex coords [18, pair_cap]
    pair_cap = cfg["pair_cap"]
    n_dev_pairs = min(npairs, pair_cap)
    planes = np.zeros((18, pair_cap), np_f32)
    if n_dev_pairs > 0:
        va = verts[faces[fa[:n_dev_pairs]]]     # [n,3(vert),3(xyz)]
        vb = verts[faces[fb[:n_dev_pairs]]]
        planes[0:9, :n_dev_pairs] = va.reshape(n_dev_pairs, 9).T
        planes[9:18, :n_dev_pairs] = vb.reshape(n_dev_pairs, 9).T
    # leftover pairs (beyond device capacity) handled on host
    edge_extra = 0.0
    if npairs > pair_cap:
        va = verts[faces[fa[pair_cap:]]]
        vb = verts[faces[fb[pair_cap:]]]
        na = np.cross(va[:, 1] - va[:, 0], va[:, 2] - va[:, 0])
        nb = np.cross(vb[:, 1] - vb[:, 0], vb[:, 2] - vb[:, 0])
        na /= np.maximum(np.linalg.norm(na, axis=-1, keepdims=True), 1e-12)
        nb /= np.maximum(np.linalg.norm(nb, axis=-1, keepdims=True), 1e-12)
        cos = (na * nb).sum(-1)
        edge_extra = float(np.maximum(cos - DIH_THR, 0.0).sum())

    pair_f = cfg["pair_f"]
    planes8 = planes.reshape(18, N_CORES, 128 * pair_f).transpose(1, 0, 2) \
                    .reshape(N_CORES, 18, 128, pair_f)

    in_maps = []
    for c in range(N_CORES):
        pr_sh = pad_pts(pred_pts[c * shard:(c + 1) * shard], rows_pad)

        sp = np.zeros(128 * sdf_f, np_f32)
        sg = np.zeros(128 * sdf_f, np_f32)
        sl = pred_sdf[c * sdf_shard:(c + 1) * sdf_shard]
        sp[:sl.shape[0]] = sl
        sg[:sl.shape[0]] = gt_sdf[c * sdf_shard:(c + 1) * sdf_shard]

        # eikonal: core covers flat diff positions [c*sdf_shard, c*sdf_shard+128*eik_f)
        ep = np.zeros(128 * eik_f + 1, np_f32)
        src = pred_sdf[c * sdf_shard: c * sdf_shard + 128 * eik_f + 1]
        ep[:src.shape[0]] = src
        eg = np.full(128 * eik_f, 1e9, np_f32)
        gsrc = gt_sdf[c * sdf_shard: c * sdf_shard + 128 * eik_f]
        eg[:gsrc.shape[0]] = gsrc
        # invalidate: local >= sdf_shard ; global seam positions j==n_batch-1 mod n_batch
        locs = np.arange(128 * eik_f)
        glob = locs + c * sdf_shard
        bad_m = (locs >= sdf_shard) | ((glob % n_batch) == n_batch - 1) | \
                (glob >= n_tot - 1)
        eg[bad_m] = 1e9

        in_maps.append({
            "a_pred": np.ascontiguousarray(a_aug(pr_sh)),
            "b_gt": b_gt_full,
            "sdf_pred": sp.reshape(128, sdf_f),
            "sdf_gt": sg.reshape(128, sdf_f),
            "eik_pred": ep,
            "eik_gt": eg.reshape(128, eik_f),
            "edge_in": np.ascontiguousarray(planes8[c]),
        })

    meta = dict(npairs=npairs, wt=wt, edge_extra=edge_extra, shard=shard)
    return in_maps, meta


def _host_post(inputs, cfg, results, meta):
    npts = cfg["npts"]
    shard = meta["shard"]
    rows_pad = cfg["rows_pad"]
    n_strips = rows_pad // 128

    rowmaxA = np.empty(npts, np.float64)
    idxA = np.empty(npts, np.int64)
    for c in range(N_CORES):
        cham = results[c]["cham_out"]  # [128, 2*ns]
        # (p, s) -> local row s*128+p
        rmA = cham[:, 0:n_strips].T.reshape(-1)[:shard]
        ixA = cham[:, n_strips:2 * n_strips].T.reshape(-1)[:shard]
        rowmaxA[c * shard:(c + 1) * shard] = rmA
        idxA[c * shard:(c + 1) * shard] = ixA.astype(np.int64)

    minA = -rowmaxA
    # colmax partials: [128 col-within-chunk, n_chunk] per core; col j of the
    # global matrix lives at [j % 128, j // 128]
    cm = np.stack([results[c]["colmax_out"] for c in range(N_CORES)])  # [8,128,nch]
    colmax = cm.max(axis=0).T.reshape(-1)[:npts]                       # [10000]
    minB = -colmax.astype(np.float64)
    ch = minA.mean() + minB.mean()

    # normal consistency (host gather + cosine over 10000 rows)
    pn = inputs["pred_normals"][0].astype(np.float64)
    gn = inputs["gt_normals"][0].astype(np.float64)
    idxA = np.clip(idxA, 0, npts - 1)
    matched = gn[idxA]
    eps = 1e-8
    num = (pn * matched).sum(-1)
    den = np.maximum(np.linalg.norm(pn, axis=-1), eps) * \
        np.maximum(np.linalg.norm(matched, axis=-1), eps)
    nrm = float(np.mean(1.0 - np.abs(num / den)))

    parts = np.stack([results[c]["part_out"] for c in range(N_CORES)])  # [8,128,8]
    psum = parts.astype(np.float64).sum(axis=(0, 1))                    # [8]
    sdf = (psum[0] + psum[1]) / float(cfg["sdf_n"])
    eik = (psum[2] / psum[3]) if psum[3] > 0 else 0.0

    npairs = meta["npairs"]
    edge = ((psum[4] + meta["edge_extra"]) / npairs) if npairs > 0 else 0.0

    total = (SDF_W * sdf + EIK_W * eik + CH_W * ch + NORM_W * nrm +
             EDGE_W * edge + WT_W * meta["wt"])
    return np.asarray(np.float32(total))


def kernel(**inputs):
    from concourse.bass_utils import run_bass_kernel_spmd
    cfg = FULL_CFG
    nc = get_program()
    in_maps, meta = _host_prep(inputs, cfg)
    res = run_bass_kernel_spmd(nc, in_maps, core_ids=list(range(N_CORES)))
    return _host_post(inputs, cfg, res.results, meta)
